# revision 11
# baseline (speedup 1.0000x reference)
"""Cox partial-likelihood (DeepSurv) loss on 8 TRN2 NeuronCores.

Math: P_exp_sum[i] = sum_j P_exp[j] * (T[i] < T[j]); loss is the
Ef-weighted mean of -log(clip(P_exp / (P_exp_sum + eps), eps, max)).

The risk-set matrix M[i,j] = (T[i] < T[j]) is (up to ties) a
permutation of a strictly-upper-triangular matrix: in T-ascending
order the risk-set sum is a strict suffix sum of the sorted P_exp.
The host argsorts T (the previous full-mask kernel already relied on a
host-side sort via np.unique for its tie correction); the device then
computes the entire [N,N]-equivalent risk-set reduction AND the loss
epilogue exactly, data-parallel over 2048 sorted rows per core:

- rows are grouped into 128 blocks of 128 (16 blocks per core);
- within-block strict suffix sums: one [128,128] strictly-triangular
  matmul per core (stationary = the core's 16 P columns, moving = the
  triangular ones matrix);
- cross-block suffix: DVE reduces per-block totals, a [128,1]x[128,16]
  matmul forms per-block suffix sums, and a K=1 matmul broadcast-
  accumulates them into the same PSUM tile;
- exact tie handling: a host-computed per-row offset (EPS - corr_i,
  corr_i = sum of P_exp over later-sorted ties of i) is added on DVE,
  so the device result is G_i + EPS with strict-< semantics;
- epilogue on device: -log(P_clipped)*Ef per row via ACT Ln (ln P_exp
  == P_risk exactly, so -ln(P_tmp) = ln(G+eps) - P_risk; the lower
  clip at EPS becomes min(., -ln EPS); the upper clip at max(P_tmp) is
  a value no-op), reduced to a per-core partial numerator with a final
  ones-matmul over partitions. The host sums the 8 partial scalars and
  divides by sum(Ef).
"""

import numpy as np

N = 16384
NCORES = 8
NBLK = 128            # sorted-row blocks of 128
BPC = NBLK // NCORES  # blocks per core = 16
LI = N // NCORES      # rows per core = 2048
EPS = 1e-6
NEG_LN_EPS = float(-np.log(np.float32(EPS)))

# packed-input column layout (all fp32, [128, XC])
_C_PB = 0      # [128,128] pe_byblock[b, j] = P_s[b*128 + j]
_C_U1 = 128    # [128,128] U1[c, i] = 1.0 if c > i
_C_PC = 256    # [128,16]  pcore[p, k] = P_s[(blk0+k)*128 + p]
_C_UC = 272    # [128,16]  Ucore[b, k] = 1.0 if b > blk0 + k
_C_OF = 288    # [128,16]  off[p, k] = EPS - corr[(blk0+k)*128 + p]
_C_EF = 304    # [128,16]  Ef_s per core, same layout as pcore
_C_PR = 320    # [128,16]  P_risk_s per core, same layout
_C_OC = 336    # [128,1]   ones column
_C_OR = 337    # [128,128] ones (row 0 used as the K=1 broadcast lhsT)
_C_ID = 465    # [128,128] identity (adds the per-row offset via the PE)
XC = 593
NOUT = 18      # out cols: 0:16 g_eps, 16 row-partials, 17 scalar (row 0)

_prog_cache = {}


def _build_program(reps=1):
    if reps in _prog_cache:
        return _prog_cache[reps]
    import concourse.bacc as bacc
    import concourse.tile as tile
    import concourse.mybir as mybir

    f32 = mybir.dt.float32
    nc = bacc.Bacc(
        "TRN2", target_bir_lowering=False, debug=False, num_devices=NCORES
    )
    inp = nc.dram_tensor("inp", [128, XC], f32, kind="ExternalInput").ap()
    out = nc.dram_tensor("out", [128, NOUT], f32, kind="ExternalOutput").ap()

    with tile.TileContext(nc) as tc:
        with (
            tc.tile_pool(name="const", bufs=1) as cpool,
            tc.tile_pool(name="work", bufs=3) as wpool,
            tc.tile_pool(name="psa", bufs=2, space="PSUM") as pa,
            tc.tile_pool(name="psb", bufs=2, space="PSUM") as pb,
            tc.tile_pool(name="psc", bufs=2, space="PSUM") as pc,
        ):
            inp_s = cpool.tile([128, XC], f32)
            nc.sync.dma_start(inp_s[:], inp[:])
            res = cpool.tile([128, NOUT], f32)
            nc.vector.memset(res[:, 17:18], 0.0)

            def head():
                # per-block totals -> S_suf
                totals = wpool.tile([128, 1], f32, name="totals", tag="tot")
                nc.vector.tensor_reduce(
                    totals[:],
                    inp_s[:, _C_PB : _C_PB + 128],
                    mybir.AxisListType.X,
                    mybir.AluOpType.add,
                )
                sr_ps = pa.tile([1, BPC], f32, name="sr_ps", tag="srp")
                nc.tensor.matmul(
                    sr_ps[:],
                    totals[:],
                    inp_s[:, _C_UC : _C_UC + BPC],
                    start=True,
                    stop=True,
                )
                srow = wpool.tile([1, BPC], f32, name="srow", tag="srow")
                nc.vector.tensor_copy(srow[:], sr_ps[:])
                # within-block strict suffix + per-row offset (identity
                # matmul folds EPS and the tie corr) + S_suf broadcast,
                # accumulated in one PSUM group
                g_ps = pb.tile([128, BPC], f32, name="g_ps", tag="gp")
                nc.tensor.matmul(
                    g_ps[:],
                    inp_s[:, _C_U1 : _C_U1 + 128],
                    inp_s[:, _C_PC : _C_PC + BPC],
                    start=True,
                    stop=False,
                )
                nc.tensor.matmul(
                    g_ps[:],
                    inp_s[:, _C_ID : _C_ID + 128],
                    inp_s[:, _C_OF : _C_OF + BPC],
                    start=False,
                    stop=False,
                )
                nc.tensor.matmul(
                    g_ps[:],
                    inp_s[0:1, _C_OR : _C_OR + 128],
                    srow[:],
                    start=False,
                    stop=True,
                )
                # -ln(P_tmp) = ln(G+eps) - P_risk (ACT reads PSUM)
                lng = wpool.tile([128, BPC], f32, name="lng", tag="lng")
                nc.scalar.activation(
                    lng[:], g_ps[:], mybir.ActivationFunctionType.Ln
                )
                return g_ps, lng

            def tail(lng):
                d = wpool.tile([128, BPC], f32, name="d", tag="d")
                nc.vector.tensor_tensor(
                    d[:], lng[:], inp_s[:, _C_PR : _C_PR + BPC],
                    mybir.AluOpType.subtract,
                )
                dc = wpool.tile([128, BPC], f32, name="dc", tag="dc")
                nc.vector.tensor_scalar_min(dc[:], d[:], NEG_LN_EPS)
                mt = wpool.tile([128, BPC], f32, name="mt", tag="mt")
                nc.vector.tensor_tensor(
                    mt[:], dc[:], inp_s[:, _C_EF : _C_EF + BPC],
                    mybir.AluOpType.mult,
                )
                rs = wpool.tile([128, 1], f32, name="rs", tag="rs")
                nc.vector.tensor_reduce(
                    rs[:], mt[:], mybir.AxisListType.X, mybir.AluOpType.add,
                )
                sc_ps = pc.tile([1, 1], f32, name="sc_ps", tag="sc")
                nc.tensor.matmul(
                    sc_ps[:],
                    rs[:],
                    inp_s[:, _C_OC : _C_OC + 1],
                    start=True,
                    stop=True,
                )
                return rs, sc_ps

            # software-pipelined: rep r's post-Ln tail is emitted during
            # rep r+1 so no engine queue head-waits on a fresh cross-
            # engine result
            g_ps = lng = None
            for r in range(reps):
                prev_lng = lng
                g_ps, lng = head()
                if prev_lng is not None:
                    tail(prev_lng)
            rs, sc_ps = tail(lng)
            # export the last rep's results (constant cost, outside the body)
            nc.vector.tensor_copy(res[:, 0:16], g_ps[:])
            nc.vector.tensor_copy(res[:, 16:17], rs[:])
            nc.vector.tensor_copy(res[0:1, 17:18], sc_ps[:])
            nc.sync.dma_start(out[:], res[:])
    nc.compile()
    _prog_cache[reps] = nc
    return nc


def _tie_corr(T_s, P_s):
    """corr[i] = sum of P_s over later-sorted j with T_s[j] == T_s[i]
    (the device's index-strict suffix overcounts exactly this)."""
    corr = np.zeros(N, np.float32)
    neq = T_s[1:] != T_s[:-1]
    if neq.all():
        return corr
    starts = np.flatnonzero(np.concatenate(([True], neq)))
    lens = np.diff(np.append(starts, N))
    for st, ln in zip(starts[lens > 1], lens[lens > 1]):
        g = P_s[st : st + ln].astype(np.float64)
        sfx = np.cumsum(g[::-1])[::-1] - g
        corr[st : st + ln] = sfx.astype(np.float32)
    return corr


def _make_in_maps(P_risk, T, E):
    P_risk = P_risk.astype(np.float32)
    T = T.astype(np.float32)
    P_exp = np.exp(P_risk)
    Ef = E.astype(np.float32) * (T < T.max()).astype(np.float32)

    order = np.argsort(T, kind="stable")
    T_s = T[order]
    P_s = P_exp[order]
    Pr_s = P_risk[order]
    Ef_s = Ef[order]
    corr = _tie_corr(T_s, P_s)
    offv = np.float32(EPS) - corr

    pe_byblock = np.ascontiguousarray(P_s.reshape(NBLK, 128))
    u1 = np.greater.outer(np.arange(128), np.arange(128)).astype(np.float32)
    onescol = np.ones((128, 1), np.float32)
    onesrow = np.ones((128, 128), np.float32)
    ident = np.eye(128, dtype=np.float32)

    def core_cols(v):  # sorted [N] -> per-core [128, BPC]
        return np.ascontiguousarray(v.reshape(NBLK, 128).T)

    pc_all = P_s.reshape(NBLK, 128)
    of_all = offv.reshape(NBLK, 128)
    ef_all = Ef_s.reshape(NBLK, 128)
    pr_all = Pr_s.reshape(NBLK, 128)

    in_maps = []
    for c in range(NCORES):
        b0 = c * BPC
        uc = np.greater.outer(
            np.arange(128), b0 + np.arange(BPC)
        ).astype(np.float32)
        blk = slice(b0, b0 + BPC)
        inp = np.concatenate(
            [
                pe_byblock,
                u1,
                np.ascontiguousarray(pc_all[blk].T),
                uc,
                np.ascontiguousarray(of_all[blk].T),
                np.ascontiguousarray(ef_all[blk].T),
                np.ascontiguousarray(pr_all[blk].T),
                onescol,
                onesrow,
                ident,
            ],
            axis=1,
        )
        assert inp.shape == (128, XC) and inp.dtype == np.float32
        in_maps.append({"inp": inp})

    aux = {
        "P_exp": P_exp,
        "order": order,
        "corr": corr,
        "Ef": Ef,
        "P_s": P_s,
    }
    return in_maps, aux


def kernel(P_risk, T, E):
    from concourse.bass_utils import run_bass_kernel_spmd

    nc = _build_program()
    in_maps, aux = _make_in_maps(P_risk, T, E)
    denom = np.sum(aux["Ef"], dtype=np.float32)
    S_total = float(aux["P_exp"].sum(dtype=np.float64))
    last_err = None
    for _attempt in range(3):
        try:
            res = run_bass_kernel_spmd(nc, in_maps, core_ids=list(range(NCORES)))
            outs = np.stack([res.results[c]["out"] for c in range(NCORES)])
            partials = outs[:, 0, 17]
            # g_eps back to sorted order: core c col k row p -> (c*16+k)*128+p
            g_eps = np.transpose(outs[:, :, 0:16], (0, 2, 1)).reshape(N)
            s_dev = g_eps.astype(np.float64) - EPS + aux["corr"]
            # sanity: suffix sums are non-increasing in sorted order, start
            # near S_total, and the max-T row has an empty risk set.
            ok = (
                np.isfinite(outs).all()
                and float(np.max(np.diff(s_dev))) < 0.5
                and abs(s_dev[0] + aux["P_s"][0] - S_total) < 0.005 * S_total
                and abs(s_dev[-1]) < 1e-2
                and s_dev.min() > -1e-2
            )
            if ok:
                loss = np.float32(partials.sum(dtype=np.float64)) / denom
                return np.asarray(loss, dtype=np.float32)
            last_err = RuntimeError("device output failed sanity check")
        except Exception as e:  # transient NRT device errors happen
            last_err = e
    raise last_err


# revision 14
# speedup vs baseline: 1.1788x; 1.1788x over previous
"""Cox partial-likelihood (DeepSurv) loss on 8 TRN2 NeuronCores.

Math: P_exp_sum[i] = sum_j P_exp[j] * (T[i] < T[j]); loss is the
Ef-weighted mean of -log(clip(P_exp / (P_exp_sum + eps), eps, max)).

The risk-set matrix M[i,j] = (T[i] < T[j]) is (up to ties) a
permutation of a strictly-upper-triangular matrix: in T-ascending
order the risk-set sum is a strict suffix sum of the sorted P_exp.
The host argsorts T (the previous full-mask kernel already relied on a
host-side sort via np.unique for its tie correction); the device then
computes the entire [N,N]-equivalent risk-set reduction AND the loss
epilogue exactly, data-parallel over 2048 sorted rows per core:

- rows are grouped into 128 blocks of 128 (16 blocks per core);
- within-block strict suffix sums: one [128,128] strictly-triangular
  matmul per core (stationary = the core's 16 P columns, moving = the
  triangular ones matrix);
- cross-block suffix: DVE reduces per-block totals, a [128,1]x[128,16]
  matmul forms per-block suffix sums, and a K=1 matmul broadcast-
  accumulates them into the same PSUM tile;
- exact tie handling: a host-computed per-row offset (EPS - corr_i,
  corr_i = sum of P_exp over later-sorted ties of i) is added on DVE,
  so the device result is G_i + EPS with strict-< semantics;
- epilogue on device: -log(P_clipped)*Ef per row via ACT Ln (ln P_exp
  == P_risk exactly, so -ln(P_tmp) = ln(G+eps) - P_risk; the lower
  clip at EPS becomes min(., -ln EPS); the upper clip at max(P_tmp) is
  a value no-op), reduced to a per-core partial numerator with a final
  ones-matmul over partitions. The host sums the 8 partial scalars and
  divides by sum(Ef).
"""

import numpy as np

N = 16384
NCORES = 8
NBLK = 128            # sorted-row blocks of 128
BPC = NBLK // NCORES  # blocks per core = 16
LI = N // NCORES      # rows per core = 2048
EPS = 1e-6
NEG_LN_EPS = float(-np.log(np.float32(EPS)))

# packed-input column layout (all fp32, [128, XC])
_C_PB = 0      # [128,128] pe_byblock[b, j] = P_s[b*128 + j]
_C_U1 = 128    # [128,128] U1[c, i] = 1.0 if c > i
_C_PC = 256    # [128,16]  pcore[p, k] = P_s[(blk0+k)*128 + p]
_C_UC = 272    # [128,16]  Ucore[b, k] = 1.0 if b > blk0 + k
_C_OF = 288    # [128,16]  off[p, k] = EPS - corr[(blk0+k)*128 + p]
_C_EF = 304    # [128,16]  Ef_s per core, same layout as pcore
_C_PR = 320    # [128,16]  P_risk_s per core, same layout
_C_OC = 336    # [128,1]   ones column
_C_OR = 337    # [128,128] ones (row 0 used as the K=1 broadcast lhsT)
_C_ID = 465    # [128,128] identity (adds the per-row offset via the PE)
XC = 593
NOUT = 18      # out cols: 0:16 g_eps, 16 row-partials, 17 scalar (row 0)

_prog_cache = {}


def _build_program(reps=1):
    if reps in _prog_cache:
        return _prog_cache[reps]
    import concourse.bacc as bacc
    import concourse.tile as tile
    import concourse.mybir as mybir

    f32 = mybir.dt.float32
    nc = bacc.Bacc(
        "TRN2", target_bir_lowering=False, debug=False, num_devices=NCORES
    )
    inp = nc.dram_tensor("inp", [128, XC], f32, kind="ExternalInput").ap()
    out = nc.dram_tensor("out", [128, NOUT], f32, kind="ExternalOutput").ap()

    with tile.TileContext(nc) as tc:
        with (
            tc.tile_pool(name="const", bufs=1) as cpool,
            tc.tile_pool(name="work", bufs=3) as wpool,
            tc.tile_pool(name="psa", bufs=2, space="PSUM") as pa,
            tc.tile_pool(name="psb", bufs=2, space="PSUM") as pb,
            tc.tile_pool(name="psc", bufs=2, space="PSUM") as pc,
        ):
            inp_s = cpool.tile([128, XC], f32)
            nc.sync.dma_start(inp_s[:], inp[:])
            res = cpool.tile([128, NOUT], f32)
            nc.vector.memset(res[:, 17:18], 0.0)

            def head():
                # per-block totals -> S_suf
                totals = wpool.tile([128, 1], f32, name="totals", tag="tot")
                nc.vector.tensor_reduce(
                    totals[:],
                    inp_s[:, _C_PB : _C_PB + 128],
                    mybir.AxisListType.X,
                    mybir.AluOpType.add,
                )
                sr_ps = pa.tile([1, BPC], f32, name="sr_ps", tag="srp")
                nc.tensor.matmul(
                    sr_ps[:],
                    totals[:],
                    inp_s[:, _C_UC : _C_UC + BPC],
                    start=True,
                    stop=True,
                )
                srow = wpool.tile([1, BPC], f32, name="srow", tag="srow")
                nc.vector.tensor_copy(srow[:], sr_ps[:])
                # within-block strict suffix + S_suf broadcast, one PSUM group
                g_ps = pb.tile([128, BPC], f32, name="g_ps", tag="gp")
                nc.tensor.matmul(
                    g_ps[:],
                    inp_s[:, _C_U1 : _C_U1 + 128],
                    inp_s[:, _C_PC : _C_PC + BPC],
                    start=True,
                    stop=False,
                )
                nc.tensor.matmul(
                    g_ps[:],
                    inp_s[0:1, _C_OR : _C_OR + 128],
                    srow[:],
                    start=False,
                    stop=True,
                )
                # g_eps = G + EPS (off folds EPS and the exact tie corr)
                ge = wpool.tile([128, BPC], f32, name="ge", tag="ge")
                nc.vector.tensor_tensor(
                    ge[:], g_ps[:], inp_s[:, _C_OF : _C_OF + BPC],
                    mybir.AluOpType.add,
                )
                # -ln(P_tmp) = ln(G+eps) - P_risk
                lng = wpool.tile([128, BPC], f32, name="lng", tag="lng")
                nc.scalar.activation(
                    lng[:], ge[:], mybir.ActivationFunctionType.Ln
                )
                return ge, lng

            def tail(lng):
                d = wpool.tile([128, BPC], f32, name="d", tag="d")
                nc.vector.tensor_tensor(
                    d[:], lng[:], inp_s[:, _C_PR : _C_PR + BPC],
                    mybir.AluOpType.subtract,
                )
                dc = wpool.tile([128, BPC], f32, name="dc", tag="dc")
                nc.vector.tensor_scalar_min(dc[:], d[:], NEG_LN_EPS)
                mt = wpool.tile([128, BPC], f32, name="mt", tag="mt")
                nc.vector.tensor_tensor(
                    mt[:], dc[:], inp_s[:, _C_EF : _C_EF + BPC],
                    mybir.AluOpType.mult,
                )
                rs = wpool.tile([128, 1], f32, name="rs", tag="rs")
                nc.vector.tensor_reduce(
                    rs[:], mt[:], mybir.AxisListType.X, mybir.AluOpType.add,
                )
                sc_ps = pc.tile([1, 1], f32, name="sc_ps", tag="sc")
                nc.tensor.matmul(
                    sc_ps[:],
                    rs[:],
                    inp_s[:, _C_OC : _C_OC + 1],
                    start=True,
                    stop=True,
                )
                return rs, sc_ps

            # software-pipelined: rep r's post-Ln tail is emitted during
            # rep r+1 so no engine queue head-waits on a fresh cross-
            # engine result
            ge = lng = None
            for r in range(reps):
                prev_lng = lng
                ge, lng = head()
                if prev_lng is not None:
                    tail(prev_lng)
            rs, sc_ps = tail(lng)
            # export the last rep's results (constant cost, outside the body)
            nc.vector.tensor_copy(res[:, 0:16], ge[:])
            nc.vector.tensor_copy(res[:, 16:17], rs[:])
            nc.vector.tensor_copy(res[0:1, 17:18], sc_ps[:])
            nc.sync.dma_start(out[:], res[:])
    nc.compile()
    _prog_cache[reps] = nc
    return nc


def _tie_corr(T_s, P_s):
    """corr[i] = sum of P_s over later-sorted j with T_s[j] == T_s[i]
    (the device's index-strict suffix overcounts exactly this)."""
    corr = np.zeros(N, np.float32)
    neq = T_s[1:] != T_s[:-1]
    if neq.all():
        return corr
    starts = np.flatnonzero(np.concatenate(([True], neq)))
    lens = np.diff(np.append(starts, N))
    for st, ln in zip(starts[lens > 1], lens[lens > 1]):
        g = P_s[st : st + ln].astype(np.float64)
        sfx = np.cumsum(g[::-1])[::-1] - g
        corr[st : st + ln] = sfx.astype(np.float32)
    return corr


def _make_in_maps(P_risk, T, E):
    P_risk = P_risk.astype(np.float32)
    T = T.astype(np.float32)
    P_exp = np.exp(P_risk)
    Ef = E.astype(np.float32) * (T < T.max()).astype(np.float32)

    order = np.argsort(T, kind="stable")
    T_s = T[order]
    P_s = P_exp[order]
    Pr_s = P_risk[order]
    Ef_s = Ef[order]
    corr = _tie_corr(T_s, P_s)
    offv = np.float32(EPS) - corr

    pe_byblock = np.ascontiguousarray(P_s.reshape(NBLK, 128))
    u1 = np.greater.outer(np.arange(128), np.arange(128)).astype(np.float32)
    onescol = np.ones((128, 1), np.float32)
    onesrow = np.ones((128, 128), np.float32)
    ident = np.eye(128, dtype=np.float32)

    def core_cols(v):  # sorted [N] -> per-core [128, BPC]
        return np.ascontiguousarray(v.reshape(NBLK, 128).T)

    pc_all = P_s.reshape(NBLK, 128)
    of_all = offv.reshape(NBLK, 128)
    ef_all = Ef_s.reshape(NBLK, 128)
    pr_all = Pr_s.reshape(NBLK, 128)

    in_maps = []
    for c in range(NCORES):
        b0 = c * BPC
        uc = np.greater.outer(
            np.arange(128), b0 + np.arange(BPC)
        ).astype(np.float32)
        blk = slice(b0, b0 + BPC)
        inp = np.concatenate(
            [
                pe_byblock,
                u1,
                np.ascontiguousarray(pc_all[blk].T),
                uc,
                np.ascontiguousarray(of_all[blk].T),
                np.ascontiguousarray(ef_all[blk].T),
                np.ascontiguousarray(pr_all[blk].T),
                onescol,
                onesrow,
                ident,
            ],
            axis=1,
        )
        assert inp.shape == (128, XC) and inp.dtype == np.float32
        in_maps.append({"inp": inp})

    aux = {
        "P_exp": P_exp,
        "order": order,
        "corr": corr,
        "Ef": Ef,
        "P_s": P_s,
    }
    return in_maps, aux


def kernel(P_risk, T, E):
    from concourse.bass_utils import run_bass_kernel_spmd

    nc = _build_program()
    in_maps, aux = _make_in_maps(P_risk, T, E)
    denom = np.sum(aux["Ef"], dtype=np.float32)
    S_total = float(aux["P_exp"].sum(dtype=np.float64))
    last_err = None
    for _attempt in range(3):
        try:
            res = run_bass_kernel_spmd(nc, in_maps, core_ids=list(range(NCORES)))
            outs = np.stack([res.results[c]["out"] for c in range(NCORES)])
            partials = outs[:, 0, 17]
            # g_eps back to sorted order: core c col k row p -> (c*16+k)*128+p
            g_eps = np.transpose(outs[:, :, 0:16], (0, 2, 1)).reshape(N)
            s_dev = g_eps.astype(np.float64) - EPS + aux["corr"]
            # sanity: suffix sums are non-increasing in sorted order, start
            # near S_total, and the max-T row has an empty risk set.
            ok = (
                np.isfinite(outs).all()
                and float(np.max(np.diff(s_dev))) < 0.5
                and abs(s_dev[0] + aux["P_s"][0] - S_total) < 0.005 * S_total
                and abs(s_dev[-1]) < 1e-2
                and s_dev.min() > -1e-2
            )
            if ok:
                loss = np.float32(partials.sum(dtype=np.float64)) / denom
                return np.asarray(loss, dtype=np.float32)
            last_err = RuntimeError("device output failed sanity check")
        except Exception as e:  # transient NRT device errors happen
            last_err = e
    raise last_err
